# revision 18
# baseline (speedup 1.0000x reference)
"""Trainium2 Bass kernel for nn_DMlp_46823733461564 (dense_mlp).

Computes: token-grid 3x3 masked-neighborhood gather (pixel-shuffle +
reflection-pad + masked unfold, algebraically reduced to a channel-
permuted shifted gather) followed by fc1(1600->1024) + exact GELU +
fc2(1024->576).

Sharding: data-parallel over (batch, image-half) -> 8 cores, 8192 tokens
each; fc weights replicated. The gather runs on-device as strided DMAs
from a host-prepared reflection-extended channel-major image; matmuls run
in bf16 on the PE (fp32 PSUM accumulation), ~3e-3 relative error.

Startup is DMA-paced: w1 K-chunks are loaded j-ordered across two queues
and tile-0 feat chunks on a third, so fc1 starts as soon as chunk 0
lands; later tiles use one merged gather DMA each.
"""
import os
import sys

import numpy as np

_TRN_REPO = "/opt/trn_rl_repo"
if _TRN_REPO not in sys.path:
    sys.path.insert(0, _TRN_REPO)

B, HIMG, WIMG = 4, 128, 128
C = 64
L = 576           # C * 9
NTOK = HIMG * WIMG
HID = 1024
OUTF = 576
INF = 1600        # C * 25
N_CORES = 8
ROWS_PER_CORE = HIMG // 2          # 64 token rows
TOK_PER_CORE = ROWS_PER_CORE * WIMG  # 8192
TILE_ROWS = 4                      # image rows per token tile
TT = TILE_ROWS * WIMG              # 512 tokens per tile
N_TILES = ROWS_PER_CORE // TILE_ROWS  # 16
KC = 13                            # ceil(25/2) K-chunks of (up to) 128

_MASK = np.array([
    [1, 0, 0, 1, 0, 0, 1],
    [0, 1, 0, 1, 0, 1, 0],
    [0, 0, 1, 1, 1, 0, 0],
    [1, 1, 1, 1, 1, 1, 1],
    [0, 0, 1, 1, 1, 0, 0],
    [0, 1, 0, 1, 0, 1, 0],
    [1, 0, 0, 1, 0, 0, 1]], dtype=bool)
MASK_POS = [(i, j) for i in range(7) for j in range(7) if _MASK[i, j]]


def _dmap(d):
    if d <= 1:
        return -1, d + 1
    if d <= 4:
        return 0, d - 2
    return 1, d - 5


KPOS = []
for (_di, _dj) in MASK_POS:
    _dh, _r1 = _dmap(_di)
    _dw, _r2 = _dmap(_dj)
    KPOS.append((_dh, _dw, _r1 * 3 + _r2))


def _swap_map(a, b, which):
    ch = np.arange(L)
    c, rem = ch // 9, ch % 9
    r1, r2 = rem // 3, rem % 3
    r = r1 if which == 0 else r2
    rs = np.where(r == a, b, np.where(r == b, a, r))
    if which == 0:
        return c * 9 + rs * 3 + r2
    return c * 9 + r1 * 3 + rs


def _build_xe(x):
    """x: (B, NTOK, L) -> xe: (B, L, HIMG+2, WIMG+2) reflection-extended,
    channel-permuted borders."""
    xt = np.ascontiguousarray(x.transpose(0, 2, 1)).reshape(B, L, HIMG, WIMG)
    xe = np.empty((B, L, HIMG + 2, WIMG + 2), dtype=np.float32)
    xe[:, :, 1:-1, 1:-1] = xt
    xe[:, :, 0, 1:-1] = xt[:, _swap_map(1, 2, 0), 0, :]
    xe[:, :, -1, 1:-1] = xt[:, _swap_map(0, 1, 0), -1, :]
    xe[:, :, :, 0] = np.take(xe[:, :, :, 1], _swap_map(1, 2, 1), axis=1)
    xe[:, :, :, -1] = np.take(xe[:, :, :, -2], _swap_map(0, 1, 1), axis=1)
    return xe


_NC_CACHE = {}

MODE = os.environ.get("KERNEL_DTYPE", "bf16")  # "f32r" | "bf16"

_WS_COUNTER = [0]


def _split_waits(nc, limit=1):
    """walrus in this toolchain accepts only ONE sync wait per instruction;
    move excess waits onto same-engine NoOps inserted just before (engine
    program order makes this equivalent)."""
    import concourse.mybir as mybir

    def noop(engine, waits):
        _WS_COUNTER[0] += 1
        return mybir.InstNoOp(
            name=f"WS-{_WS_COUNTER[0]}",
            sync_info=mybir.SyncInfo(on_wait=list(waits), on_update=[]),
            bass_nofuse=True,
            engine=engine,
        )

    for fn in nc.m.functions:
        for blk in fn.blocks:
            new_insts = []
            for inst in blk.instructions:
                si = getattr(inst, "sync_info", None)
                waits = list(si.on_wait) if si and si.on_wait else []
                if len(waits) > limit:
                    excess = waits[: len(waits) - limit]
                    si.on_wait = waits[len(waits) - limit:]
                    while excess:
                        new_insts.append(noop(inst.engine, excess[:limit]))
                        excess = excess[limit:]
                new_insts.append(inst)
            blk.instructions = new_insts


def _build_bass():
    if "nc" in _NC_CACHE:
        return _NC_CACHE["nc"]
    import concourse.bass as bass
    import concourse.mybir as mybir
    from concourse.tile import TileContext

    f32 = mybir.dt.float32
    mm_dt = mybir.dt.float32r if MODE == "f32r" else mybir.dt.bfloat16
    AF = mybir.ActivationFunctionType
    ACT = getattr(AF, os.environ.get("KERNEL_ACT", "Gelu"))
    Alu = mybir.AluOpType

    nc = bass.Bass("TRN2", target_bir_lowering=False, debug=False)
    # tile-0 rows only, chunk-major (chunk-paced cold start: 13 small DMAs)
    xp = nc.dram_tensor("xp0", (KC, 128, TILE_ROWS, WIMG), mm_dt,
                        kind="ExternalInput")
    # steady-state gather source, row-major with all 13 chunks contiguous
    # per (partition, row): one DMA per tile with ~13KB contiguous runs
    # (full DMA rate, 1 sync-queue issue) instead of 13 1KB-run DMAs
    xr = nc.dram_tensor("xr", (128, ROWS_PER_CORE, KC, WIMG), mm_dt,
                        kind="ExternalInput")
    # w1 pre-chunked (13, 128, 1024); chunk 12 rows 64:128 duplicate rows
    # 0:64 so the packed K=64 row-group matmul reads lhsT from there
    w1p = nc.dram_tensor("w1p", (KC, 128, HID), mm_dt, kind="ExternalInput")
    w2t = nc.dram_tensor("w2t", (HID // 128, 128, OUTF), mm_dt,
                         kind="ExternalInput")
    b1rs = nc.dram_tensor("b1rs", (128, HID // 128), f32, kind="ExternalInput")
    b2bc = nc.dram_tensor("b2bc", (128, OUTF), f32, kind="ExternalInput")
    out = nc.dram_tensor("out", (TOK_PER_CORE, OUTF), f32,
                         kind="ExternalOutput")

    with TileContext(nc) as tc:
        with (
            tc.tile_pool(name="wpool", bufs=1) as wpool,
            tc.tile_pool(name="fpool", bufs=3) as fpool,
            tc.tile_pool(name="hpool", bufs=2) as hpool,
            tc.tile_pool(name="opool", bufs=3) as opool,
            tc.tile_pool(name="ps1", bufs=2, space="PSUM") as ps1,
            tc.tile_pool(name="ps2", bufs=2, space="PSUM") as ps2,
        ):
            pack64 = os.environ.get("KERNEL_PACK64", "1") == "1"
            # --- PE warmup: a few dependency-free matmuls fill the gap
            # between the engine preamble (~8.4us) and the first gather/
            # weight chunk landing (~8.6us), starting the HAM busy window
            # early; real (cold-clock) fc1 matmuls then carry it to warm ---
            n_warm = int(os.environ.get("KERNEL_WARMUP", "4"))
            warm_n = int(os.environ.get("KERNEL_WARMUP_N", "128"))
            if n_warm:
                warm = wpool.tile([128, 512], mm_dt, tag="warm")
                nc.vector.memset(warm[:, :], 0.0)
                wps = ps2.tile([128, 512], f32, tag="poa")
                for _ in range(n_warm):
                    nc.tensor.matmul(wps[:, 0:warm_n], warm[:, 0:128],
                                     warm[:, 0:warm_n], start=True, stop=True)
            # --- startup loads, ordered for a DMA-paced fc1 start.
            # sync HWDGE queue: tile-0 feat chunks first (chunk-paced fc1
            # start), then the last w1 chunks; scalar HWDGE queue:
            # w1[0..8], then b1/w2/b2 (needed ~15-30us in). ---
            w1sb = []
            for j in range(KC):
                t = wpool.tile([128, HID], mm_dt, tag=f"w1_{j}")
                w1sb.append(t)
            f0 = []
            for jj in range(KC):
                ft = fpool.tile([128, TT], mm_dt, tag=f"f{jj}", bufs=1)
                f0.append(ft)
            # chunk 0 arrives as 4 row-slices (and w1[0] as 4 M-quarters
            # on the scalar queue) so the first matmul starts on the
            # first ~96KB that lands (~9.2us) instead of waiting ~3.7us
            # for a full 256KB transfer at cold-DMA rates
            for s4 in range(TILE_ROWS):
                dst = f0[0][:, s4 * WIMG : (s4 + 1) * WIMG]
                nc.sync.dma_start(out=dst, in_=xp[0, :, s4, :])
            for jj in range(1, KC):
                if jj >= 9:
                    # last w1 chunks ride the sync queue, interleaved so
                    # chunk jj's weight+feat arrive together
                    nc.sync.dma_start(out=w1sb[jj][:, :], in_=w1p[jj, :, :])
                dst = f0[jj][:, :].rearrange("p (r w) -> p r w", r=TILE_ROWS)
                nc.sync.dma_start(out=dst, in_=xp[jj, :, 0:TILE_ROWS, :])
            for q4 in range(4):
                nc.scalar.dma_start(
                    out=w1sb[0][:, q4 * 256 : (q4 + 1) * 256],
                    in_=w1p[0, :, q4 * 256 : (q4 + 1) * 256])
            for j in range(1, 9):
                nc.scalar.dma_start(out=w1sb[j][:, :], in_=w1p[j, :, :])
            b1t = wpool.tile([128, HID // 128], f32, tag="b1")
            nc.scalar.dma_start(out=b1t[:, :], in_=b1rs[:, :])
            # preload the Gelu activation table while the PE waits on the
            # first weight chunks (the table load costs 1.3us on scalar
            # and would otherwise sit in front of tile-0's GELU chain)
            if n_warm:
                # read the (memset) warm SBUF tile, NOT wps: a read of wps
                # would delay its release — and the PSUM bank that tile-0's
                # m6 accumulator reuses — behind this ACTIVATE, which the
                # scheduler parks after the scalar queue's DMA issues
                actp = wpool.tile([128, 1], f32, tag="actp")
                nc.scalar.activation(actp[:, :], warm[:, 0:1], ACT, scale=1.0)
            # fc2 weights/bias are emitted inside the tile loop (after
            # tile-1's gathers) so they don't delay tile-1 feat chunks
            # on the scalar queue
            w2sb = []
            b2t = None

            def fc2_chunk(hts, r0, s, last=False):
                # --- fc2 for one 128-token chunk: out = h.T @ w2 + b2 ---
                # N split 288+288 so both matmuls stream well past the
                # (hidden) LDWEIGHTS; each [128, 288] psum is one bank.
                # Per-half bias-add + store so the first half's output
                # DMA overlaps the second half's matmuls.
                NH = OUTF // 2
                poa = ps2.tile([128, NH], f32, tag="poa")
                pob = ps2.tile([128, NH], f32, tag="pob")
                for j in range(HID // 128):
                    nc.tensor.matmul(
                        poa[:, :],
                        hts[j][:, s * 128 : (s + 1) * 128],
                        w2sb[j][:, 0:NH],
                        start=(j == 0), stop=(j == HID // 128 - 1),
                    )
                    nc.tensor.matmul(
                        pob[:, :],
                        hts[j][:, s * 128 : (s + 1) * 128],
                        w2sb[j][:, NH:OUTF],
                        start=(j == 0), stop=(j == HID // 128 - 1),
                    )
                tok0 = (r0 * WIMG) + s * 128
                if last:
                    # final chunk: both bias-adds land in one [128, 576]
                    # tile (full 2304B DRAM rows), then two token-split
                    # stores drain on both queues in parallel — the
                    # narrow-row 90GB/s tail transfer halves
                    of = opool.tile([128, OUTF], f32, tag="of", bufs=1)
                    nc.vector.tensor_tensor(
                        out=of[:, 0:NH], in0=poa[:, :], in1=b2t[:, 0:NH],
                        op=Alu.add)
                    nc.vector.tensor_tensor(
                        out=of[:, NH:OUTF], in0=pob[:, :],
                        in1=b2t[:, NH:OUTF], op=Alu.add)
                    nc.sync.dma_start(out=out[tok0 : tok0 + 64, :],
                                      in_=of[0:64, :])
                    nc.scalar.dma_start(out=out[tok0 + 64 : tok0 + 128, :],
                                        in_=of[64:128, :])
                    return
                oa = opool.tile([128, NH], f32, tag="oa")
                nc.vector.tensor_tensor(
                    out=oa[:, :], in0=poa[:, :], in1=b2t[:, 0:NH],
                    op=Alu.add)
                nc.sync.dma_start(out=out[tok0 : tok0 + 128, 0:NH],
                                  in_=oa[:, :])
                ob = opool.tile([128, NH], f32, tag="ob")
                nc.vector.tensor_tensor(
                    out=ob[:, :], in0=pob[:, :], in1=b2t[:, NH:OUTF],
                    op=Alu.add)
                nc.sync.dma_start(out=out[tok0 : tok0 + 128, NH:OUTF],
                                  in_=ob[:, :])

            # tile list: 4-row tiles, with a 3+1 split at the end so the
            # serial fc2 tail after the final fc1 is a single 128-token
            # chunk. fc2 chunks of tile t-1 are interleaved between tile
            # t's fc1 m-pairs so the PSUM-recycle / bias-add latency
            # hides behind ~5us of fc1 instead of stalling the PE at
            # every other 2us fc2 chunk boundary.
            tiles = [(i * TILE_ROWS, TILE_ROWS) for i in range(N_TILES - 1)]
            tiles += [((N_TILES - 1) * TILE_ROWS, TILE_ROWS - 1),
                      (ROWS_PER_CORE - 1, 1)]

            prev = None  # (hts, r0, tt) of the previous tile: fc2 runs one
            # tile behind fc1 so the PE never waits on the GELU latency
            for t_i, (r0, nrows) in enumerate(tiles):
                tt = nrows * WIMG

                # tile 0's chunk DMAs were emitted up top (chunk-paced
                # fc1 start); later tiles use one merged gather DMA on
                # the sync queue (13KB contiguous runs per partition)
                if t_i == 0:
                    fts = f0

                    def ftile(j, p0=0, p1=128):
                        return fts[j][p0:p1, :]
                else:
                    fsup = fpool.tile([128, TT * KC], mm_dt, tag="fs")
                    used = fsup[:, 0 : nrows * KC * WIMG]
                    dst = used.rearrange("p (r j w) -> p r j w",
                                         r=nrows, j=KC)
                    nc.sync.dma_start(out=dst, in_=xr[:, r0 : r0 + nrows, :, :])
                    fview = used.rearrange("p (r j w) -> p j r w",
                                           r=nrows, j=KC)

                    def ftile(j, p0=0, p1=128):
                        return fview[p0:p1, j]

                if t_i == 1:
                    # w2/b2 ride the sync queue behind tile-1's gathers:
                    # the sync engine runs ~3 tiles ahead of the PE, so
                    # they land during tile 0's compute — well before the
                    # first interleaved fc2 chunk (tile-1 pair-0 end).
                    # The scalar queue would serialize them behind tile
                    # 0's eight GELU ACTIVATEs instead.
                    for j2 in range(HID // 128):
                        t = wpool.tile([128, OUTF], mm_dt, tag=f"w2_{j2}")
                        nc.sync.dma_start(out=t[:, :], in_=w2t[j2, :, :])
                        w2sb.append(t)
                    b2t = wpool.tile([128, OUTF], f32, tag="b2")
                    nc.sync.dma_start(out=b2t[:, :], in_=b2bc[:, :])
                # --- fc1 + GELU: h[m] = gelu(w1p[j].T @ featT + b1) ---
                hts = []
                if t_i == 0:
                    # tile 0 is DMA-paced: loop j OUTER with all 8 PSUM
                    # banks as accumulators, so each arriving chunk feeds
                    # 8 matmuls (~1.7us) — matching the ~1.8us/chunk HBM
                    # delivery rate instead of stalling on a chunk-starved
                    # m-pair pass
                    accs = []
                    for m in range(HID // 128):
                        pool_m = (ps1, ps2)[m // 4]
                        tag_m = ("psa", "psb")[m % 2] if m < 4 else \
                            ("poa", "pob")[m % 2]
                        acc = pool_m.tile([128, tt], f32, tag=tag_m,
                                          name=f"acc{m}")
                        accs.append(acc)
                    fine_j0 = os.environ.get("KERNEL_FINE_J0", "1") == "1"
                    if fine_j0:
                        # j=0 fine-grained: token-slice outer, m inner, so
                        # MM (s,m) only needs f-slice s + w1 quarter m//2.
                        # start=True only on s=0: a start resets the whole
                        # bank's has_written bits, so later slices must
                        # write start=False (they land on has_written=0
                        # regions and store rather than accumulate).
                        for s4 in range(4):
                            for m in range(HID // 128):
                                nc.tensor.matmul(
                                    accs[m][:, s4 * 128 : (s4 + 1) * 128],
                                    w1sb[0][:, m * 128 : (m + 1) * 128],
                                    f0[0][:, s4 * 128 : (s4 + 1) * 128],
                                    start=(s4 == 0), stop=False,
                                )
                    else:
                        for m in range(HID // 128):
                            nc.tensor.matmul(
                                accs[m][:, :],
                                w1sb[0][:, m * 128 : (m + 1) * 128],
                                ftile(0),
                                start=True, stop=False,
                            )
                    for j in range(1, KC - 1):
                        for m in range(HID // 128):
                            nc.tensor.matmul(
                                accs[m][:, :],
                                w1sb[j][:, m * 128 : (m + 1) * 128],
                                ftile(j),
                                start=False, stop=False,
                            )
                    # K=64 tail chunk: m-pairs ride PE row groups
                    # (0,0)/(64,0) concurrently, like the packed path
                    for mp0 in range(HID // 256):
                        m0, m1 = 2 * mp0, 2 * mp0 + 1
                        nc.tensor.matmul(
                            accs[m0][:, :],
                            w1sb[KC - 1][0:64, m0 * 128 : (m0 + 1) * 128],
                            ftile(KC - 1, 0, 64),
                            start=False, stop=True,
                        )
                        nc.tensor.matmul(
                            accs[m1][:, :],
                            w1sb[KC - 1][64:128, m1 * 128 : (m1 + 1) * 128],
                            ftile(KC - 1, 64, 128),
                            start=False, stop=True,
                        )
                    for m in range(HID // 128):
                        ht = hpool.tile([128, tt], mm_dt, tag=f"h{m}")
                        nc.scalar.activation(ht[:, :], accs[m][:, :], ACT,
                                             bias=b1t[:, m : m + 1], scale=1.0)
                        hts.append(ht)
                elif not pack64:
                    for m in range(HID // 128):
                        ps = ps1.tile([128, tt], f32)
                        for j in range(KC):
                            kr = 128 if j < KC - 1 else 64
                            nc.tensor.matmul(
                                ps[:, :],
                                w1sb[j][0:kr, m * 128 : (m + 1) * 128],
                                ftile(j, 0, kr),
                                start=(j == 0), stop=(j == KC - 1),
                            )
                        ht = hpool.tile([128, tt], mm_dt, tag=f"h{m}")
                        nc.scalar.activation(ht[:, :], ps[:, :], ACT,
                                             bias=b1t[:, m : m + 1], scale=1.0)
                        hts.append(ht)
                    if prev is not None:
                        ph, pr0, ptt = prev
                        for s in range(ptt // 128):
                            fc2_chunk(ph, pr0, s)
                else:
                    # chunk 12 (K=64) packed: m-pairs run their K=64 matmuls
                    # concurrently on PE row groups (0,0)/(64,0).
                    # prev tile's fc2 chunks are spread over the m-pairs
                    # (floor distribution, empty-first so chunk 0 waits
                    # for at least one ~5us pair of vector/DMA slack).
                    # The K=64 tail chunk runs as row-group (h0/h64) MM
                    # pairs, but a row-group LDWEIGHTS cannot overlap an
                    # in-flight full-array matmul (and vice versa), so
                    # every full<->row-group seam costs ~100ns of
                    # unhidden LDWEIGHTS. Group the K=64 finishes of two
                    # pairs (and the fc2 block) per seam: 2 seams/tile
                    # instead of 4.
                    npair = HID // 256
                    C = prev[2] // 128 if prev is not None else 0
                    hts = [None] * (HID // 128)
                    pend_k64 = []
                    for mp in range(npair):
                        m0, m1 = 2 * mp, 2 * mp + 1
                        psa = ps1.tile([128, tt], f32, tag="psa")
                        psb = ps1.tile([128, tt], f32, tag="psb")
                        for j in range(KC - 1):
                            nc.tensor.matmul(
                                psa[:, :], w1sb[j][:, m0 * 128:(m0 + 1) * 128],
                                ftile(j), start=(j == 0), stop=False)
                            nc.tensor.matmul(
                                psb[:, :], w1sb[j][:, m1 * 128:(m1 + 1) * 128],
                                ftile(j), start=(j == 0), stop=False)
                        pend_k64.append((mp, psa, psb))
                        if mp not in (1, 3):
                            continue
                        for mq, qa, qb in pend_k64:
                            nc.tensor.matmul(
                                qa[:, :],
                                w1sb[KC - 1][0:64, 2 * mq * 128:(2 * mq + 1) * 128],
                                ftile(KC - 1, 0, 64),
                                start=False, stop=True)
                            nc.tensor.matmul(
                                qb[:, :],
                                w1sb[KC - 1][64:128, (2 * mq + 1) * 128:(2 * mq + 2) * 128],
                                ftile(KC - 1, 64, 128),
                                start=False, stop=True)
                        for mq, qa, qb in pend_k64:
                            for m, pst in ((2 * mq, qa), (2 * mq + 1, qb)):
                                ht = hpool.tile([128, tt], mm_dt, tag=f"h{m}")
                                nc.scalar.activation(
                                    ht[:, :], pst[:, :], ACT,
                                    bias=b1t[:, m : m + 1], scale=1.0)
                                hts[m] = ht
                        pend_k64 = []
                        # fc2 chunks of the prev tile ride in blocks of 2
                        # inside the same seam window
                        if prev is not None:
                            ph, pr0, _ = prev
                            half = (C + 1) // 2
                            lo, hi = (0, half) if mp == 1 else (half, C)
                            for s in range(lo, hi):
                                fc2_chunk(ph, pr0, s)
                prev = (hts, r0, tt)
            # tail: the final tile's fc2 (a single 128-token chunk)
            ph, pr0, ptt = prev
            for s in range(ptt // 128):
                fc2_chunk(ph, pr0, s, last=(s == ptt // 128 - 1))

    if os.environ.get("KERNEL_SPLITWAITS", "1") == "1":
        _split_waits(nc)
    _NC_CACHE["nc"] = nc
    return nc


def _host_prep(x, w1, b1, w2, b2):
    x = np.ascontiguousarray(np.asarray(x, dtype=np.float32))
    w1 = np.asarray(w1, dtype=np.float32)
    b1 = np.asarray(b1, dtype=np.float32)
    w2 = np.asarray(w2, dtype=np.float32)
    b2 = np.asarray(b2, dtype=np.float32)

    xe = _build_xe(x)
    w1t = np.ascontiguousarray(w1.T)  # (1600, 1024) rows c*25+k
    w1p = np.ascontiguousarray(
        w1t.reshape(C, 25, HID).transpose(1, 0, 2).reshape(INF, HID))
    # pad to (13*128, HID): rows 1600:1664 duplicate rows 1536:1600
    w1p = np.concatenate([w1p, w1p[INF - 64 : INF]], axis=0)
    w1p = w1p.reshape(KC, 128, HID)
    w2t = np.ascontiguousarray(w2.T).reshape(HID // 128, 128, OUTF)
    b1rs = np.ascontiguousarray(b1.reshape(HID // 128, 128).T)
    b2bc = np.ascontiguousarray(np.broadcast_to(b2, (128, OUTF)))

    if MODE == "bf16":
        import ml_dtypes
        xe = xe.astype(ml_dtypes.bfloat16)
        w1p = w1p.astype(ml_dtypes.bfloat16)
        w2t = w2t.astype(ml_dtypes.bfloat16)

    in_maps = []
    for cid in range(N_CORES):
        b, half = cid // 2, cid % 2
        h0 = half * ROWS_PER_CORE
        xpair = np.empty((KC, 128, ROWS_PER_CORE, WIMG), dtype=xe.dtype)
        for j in range(KC):
            for p in range(2):
                k = min(2 * j + p, 24)
                dh, dw, q = KPOS[k]
                xpair[j, p * 64 : (p + 1) * 64] = xe[
                    b, q::9,
                    1 + h0 + dh : 1 + h0 + dh + ROWS_PER_CORE,
                    1 + dw : 1 + dw + WIMG]
        xp0 = np.ascontiguousarray(xpair[:, :, 0:TILE_ROWS, :])
        xr = np.ascontiguousarray(xpair.transpose(1, 2, 0, 3))
        in_maps.append({
            "xp0": xp0, "xr": xr, "w1p": w1p, "w2t": w2t, "b1rs": b1rs,
            "b2bc": b2bc,
        })
    return in_maps


def _assemble(results):
    out = np.empty((B, NTOK, OUTF), dtype=np.float32)
    for cid in range(N_CORES):
        b, half = cid // 2, cid % 2
        t0 = half * TOK_PER_CORE
        out[b, t0 : t0 + TOK_PER_CORE, :] = results[cid]["out"]
    return out


def kernel(x, w1, b1, w2, b2, image_h, image_w):
    in_maps = _host_prep(x, w1, b1, w2, b2)
    nc = _build_bass()
    from concourse.bass_utils import run_bass_kernel_spmd
    res = run_bass_kernel_spmd(nc, in_maps, list(range(N_CORES)))
    return _assemble(res.results)



# revision 19
# speedup vs baseline: 1.0002x; 1.0002x over previous
"""Trainium2 Bass kernel for nn_DMlp_46823733461564 (dense_mlp).

Computes: token-grid 3x3 masked-neighborhood gather (pixel-shuffle +
reflection-pad + masked unfold, algebraically reduced to a channel-
permuted shifted gather) followed by fc1(1600->1024) + exact GELU +
fc2(1024->576).

Sharding: data-parallel over (batch, image-half) -> 8 cores, 8192 tokens
each; fc weights replicated. The gather runs on-device as strided DMAs
from a host-prepared reflection-extended channel-major image; matmuls run
in bf16 on the PE (fp32 PSUM accumulation), ~3e-3 relative error.

Startup is DMA-paced: w1 K-chunks are loaded j-ordered across two queues
and tile-0 feat chunks on a third, so fc1 starts as soon as chunk 0
lands; later tiles use one merged gather DMA each.
"""
import os
import sys

import numpy as np

_TRN_REPO = "/opt/trn_rl_repo"
if _TRN_REPO not in sys.path:
    sys.path.insert(0, _TRN_REPO)

B, HIMG, WIMG = 4, 128, 128
C = 64
L = 576           # C * 9
NTOK = HIMG * WIMG
HID = 1024
OUTF = 576
INF = 1600        # C * 25
N_CORES = 8
ROWS_PER_CORE = HIMG // 2          # 64 token rows
TOK_PER_CORE = ROWS_PER_CORE * WIMG  # 8192
TILE_ROWS = 4                      # image rows per token tile
TT = TILE_ROWS * WIMG              # 512 tokens per tile
N_TILES = ROWS_PER_CORE // TILE_ROWS  # 16
KC = 13                            # ceil(25/2) K-chunks of (up to) 128

_MASK = np.array([
    [1, 0, 0, 1, 0, 0, 1],
    [0, 1, 0, 1, 0, 1, 0],
    [0, 0, 1, 1, 1, 0, 0],
    [1, 1, 1, 1, 1, 1, 1],
    [0, 0, 1, 1, 1, 0, 0],
    [0, 1, 0, 1, 0, 1, 0],
    [1, 0, 0, 1, 0, 0, 1]], dtype=bool)
MASK_POS = [(i, j) for i in range(7) for j in range(7) if _MASK[i, j]]


def _dmap(d):
    if d <= 1:
        return -1, d + 1
    if d <= 4:
        return 0, d - 2
    return 1, d - 5


KPOS = []
for (_di, _dj) in MASK_POS:
    _dh, _r1 = _dmap(_di)
    _dw, _r2 = _dmap(_dj)
    KPOS.append((_dh, _dw, _r1 * 3 + _r2))


def _swap_map(a, b, which):
    ch = np.arange(L)
    c, rem = ch // 9, ch % 9
    r1, r2 = rem // 3, rem % 3
    r = r1 if which == 0 else r2
    rs = np.where(r == a, b, np.where(r == b, a, r))
    if which == 0:
        return c * 9 + rs * 3 + r2
    return c * 9 + r1 * 3 + rs


def _build_xe(x):
    """x: (B, NTOK, L) -> xe: (B, L, HIMG+2, WIMG+2) reflection-extended,
    channel-permuted borders."""
    xt = np.ascontiguousarray(x.transpose(0, 2, 1)).reshape(B, L, HIMG, WIMG)
    xe = np.empty((B, L, HIMG + 2, WIMG + 2), dtype=np.float32)
    xe[:, :, 1:-1, 1:-1] = xt
    xe[:, :, 0, 1:-1] = xt[:, _swap_map(1, 2, 0), 0, :]
    xe[:, :, -1, 1:-1] = xt[:, _swap_map(0, 1, 0), -1, :]
    xe[:, :, :, 0] = np.take(xe[:, :, :, 1], _swap_map(1, 2, 1), axis=1)
    xe[:, :, :, -1] = np.take(xe[:, :, :, -2], _swap_map(0, 1, 1), axis=1)
    return xe


_NC_CACHE = {}

MODE = os.environ.get("KERNEL_DTYPE", "bf16")  # "f32r" | "bf16"

_WS_COUNTER = [0]


def _split_waits(nc, limit=1):
    """walrus in this toolchain accepts only ONE sync wait per instruction;
    move excess waits onto same-engine NoOps inserted just before (engine
    program order makes this equivalent)."""
    import concourse.mybir as mybir

    def noop(engine, waits):
        _WS_COUNTER[0] += 1
        return mybir.InstNoOp(
            name=f"WS-{_WS_COUNTER[0]}",
            sync_info=mybir.SyncInfo(on_wait=list(waits), on_update=[]),
            bass_nofuse=True,
            engine=engine,
        )

    for fn in nc.m.functions:
        for blk in fn.blocks:
            new_insts = []
            for inst in blk.instructions:
                si = getattr(inst, "sync_info", None)
                waits = list(si.on_wait) if si and si.on_wait else []
                if len(waits) > limit:
                    excess = waits[: len(waits) - limit]
                    si.on_wait = waits[len(waits) - limit:]
                    while excess:
                        new_insts.append(noop(inst.engine, excess[:limit]))
                        excess = excess[limit:]
                new_insts.append(inst)
            blk.instructions = new_insts


def _build_bass():
    if "nc" in _NC_CACHE:
        return _NC_CACHE["nc"]
    import concourse.bass as bass
    import concourse.mybir as mybir
    from concourse.tile import TileContext

    f32 = mybir.dt.float32
    mm_dt = mybir.dt.float32r if MODE == "f32r" else mybir.dt.bfloat16
    AF = mybir.ActivationFunctionType
    ACT = getattr(AF, os.environ.get("KERNEL_ACT", "Gelu"))
    Alu = mybir.AluOpType

    nc = bass.Bass("TRN2", target_bir_lowering=False, debug=False)
    # tile-0 rows only, chunk-major (chunk-paced cold start: 13 small DMAs)
    xp = nc.dram_tensor("xp0", (KC, 128, TILE_ROWS, WIMG), mm_dt,
                        kind="ExternalInput")
    # steady-state gather source, row-major with all 13 chunks contiguous
    # per (partition, row): one DMA per tile with ~13KB contiguous runs
    # (full DMA rate, 1 sync-queue issue) instead of 13 1KB-run DMAs
    xr = nc.dram_tensor("xr", (128, ROWS_PER_CORE, KC, WIMG), mm_dt,
                        kind="ExternalInput")
    # w1 pre-chunked (13, 128, 1024); chunk 12 rows 64:128 duplicate rows
    # 0:64 so the packed K=64 row-group matmul reads lhsT from there
    w1p = nc.dram_tensor("w1p", (KC, 128, HID), mm_dt, kind="ExternalInput")
    w2t = nc.dram_tensor("w2t", (HID // 128, 128, OUTF), mm_dt,
                         kind="ExternalInput")
    b1rs = nc.dram_tensor("b1rs", (128, HID // 128), f32, kind="ExternalInput")
    b2bc = nc.dram_tensor("b2bc", (128, OUTF), f32, kind="ExternalInput")
    out = nc.dram_tensor("out", (TOK_PER_CORE, OUTF), f32,
                         kind="ExternalOutput")

    with TileContext(nc) as tc:
        with (
            tc.tile_pool(name="wpool", bufs=1) as wpool,
            tc.tile_pool(name="fpool", bufs=3) as fpool,
            tc.tile_pool(name="hpool", bufs=2) as hpool,
            tc.tile_pool(name="opool", bufs=3) as opool,
            tc.tile_pool(name="ps1", bufs=2, space="PSUM") as ps1,
            tc.tile_pool(name="ps2", bufs=2, space="PSUM") as ps2,
        ):
            pack64 = os.environ.get("KERNEL_PACK64", "1") == "1"
            # --- PE warmup: a few dependency-free matmuls fill the gap
            # between the engine preamble (~8.4us) and the first gather/
            # weight chunk landing (~8.6us), starting the HAM busy window
            # early; real (cold-clock) fc1 matmuls then carry it to warm ---
            n_warm = int(os.environ.get("KERNEL_WARMUP", "4"))
            warm_n = int(os.environ.get("KERNEL_WARMUP_N", "128"))
            if n_warm:
                warm = wpool.tile([128, 512], mm_dt, tag="warm")
                nc.vector.memset(warm[:, :], 0.0)
                wps = ps2.tile([128, 512], f32, tag="poa")
                for _ in range(n_warm):
                    nc.tensor.matmul(wps[:, 0:warm_n], warm[:, 0:128],
                                     warm[:, 0:warm_n], start=True, stop=True)
            # --- startup loads, ordered for a DMA-paced fc1 start.
            # sync HWDGE queue: tile-0 feat chunks first (chunk-paced fc1
            # start), then the last w1 chunks; scalar HWDGE queue:
            # w1[0..8], then b1/w2/b2 (needed ~15-30us in). ---
            w1sb = []
            for j in range(KC):
                t = wpool.tile([128, HID], mm_dt, tag=f"w1_{j}")
                w1sb.append(t)
            f0 = []
            for jj in range(KC):
                ft = fpool.tile([128, TT], mm_dt, tag=f"f{jj}", bufs=1)
                f0.append(ft)
            # chunk 0 arrives as 4 row-slices (and w1[0] as 4 M-quarters
            # on the scalar queue) so the first matmul starts on the
            # first ~96KB that lands (~9.2us) instead of waiting ~3.7us
            # for a full 256KB transfer at cold-DMA rates
            for s4 in range(TILE_ROWS):
                dst = f0[0][:, s4 * WIMG : (s4 + 1) * WIMG]
                nc.sync.dma_start(out=dst, in_=xp[0, :, s4, :])
            for jj in range(1, KC):
                if jj >= 9:
                    # last w1 chunks ride the sync queue, interleaved so
                    # chunk jj's weight+feat arrive together
                    nc.sync.dma_start(out=w1sb[jj][:, :], in_=w1p[jj, :, :])
                dst = f0[jj][:, :].rearrange("p (r w) -> p r w", r=TILE_ROWS)
                nc.sync.dma_start(out=dst, in_=xp[jj, :, 0:TILE_ROWS, :])
            for q4 in range(4):
                nc.scalar.dma_start(
                    out=w1sb[0][:, q4 * 256 : (q4 + 1) * 256],
                    in_=w1p[0, :, q4 * 256 : (q4 + 1) * 256])
            for j in range(1, 9):
                nc.scalar.dma_start(out=w1sb[j][:, :], in_=w1p[j, :, :])
            b1t = wpool.tile([128, HID // 128], f32, tag="b1")
            nc.scalar.dma_start(out=b1t[:, :], in_=b1rs[:, :])
            # preload the Gelu activation table while the PE waits on the
            # first weight chunks (the table load costs 1.3us on scalar
            # and would otherwise sit in front of tile-0's GELU chain)
            if n_warm:
                # read the (memset) warm SBUF tile, NOT wps: a read of wps
                # would delay its release — and the PSUM bank that tile-0's
                # m6 accumulator reuses — behind this ACTIVATE, which the
                # scheduler parks after the scalar queue's DMA issues
                actp = wpool.tile([128, 1], f32, tag="actp")
                nc.scalar.activation(actp[:, :], warm[:, 0:1], ACT, scale=1.0)
            # fc2 weights/bias are emitted inside the tile loop (after
            # tile-1's gathers) so they don't delay tile-1 feat chunks
            # on the scalar queue
            w2sb = []
            b2t = None

            def fc2_chunk(hts, r0, s, last=False):
                # --- fc2 for one 128-token chunk: out = h.T @ w2 + b2 ---
                # N split 288+288 so both matmuls stream well past the
                # (hidden) LDWEIGHTS; each [128, 288] psum is one bank.
                # Per-half bias-add + store so the first half's output
                # DMA overlaps the second half's matmuls.
                NH = OUTF // 2
                poa = ps2.tile([128, NH], f32, tag="poa")
                pob = ps2.tile([128, NH], f32, tag="pob")
                for j in range(HID // 128):
                    nc.tensor.matmul(
                        poa[:, :],
                        hts[j][:, s * 128 : (s + 1) * 128],
                        w2sb[j][:, 0:NH],
                        start=(j == 0), stop=(j == HID // 128 - 1),
                    )
                    nc.tensor.matmul(
                        pob[:, :],
                        hts[j][:, s * 128 : (s + 1) * 128],
                        w2sb[j][:, NH:OUTF],
                        start=(j == 0), stop=(j == HID // 128 - 1),
                    )
                tok0 = (r0 * WIMG) + s * 128
                oa = opool.tile([128, NH], f32, tag="oa")
                nc.vector.tensor_tensor(
                    out=oa[:, :], in0=poa[:, :], in1=b2t[:, 0:NH],
                    op=Alu.add)
                nc.sync.dma_start(out=out[tok0 : tok0 + 128, 0:NH],
                                  in_=oa[:, :])
                ob = opool.tile([128, NH], f32, tag="ob")
                # final chunk: ob's store rides the scalar queue so the
                # two output DMAs drain in parallel after the last matmul
                eng_dma = nc.scalar if last else nc.sync
                nc.vector.tensor_tensor(
                    out=ob[:, :], in0=pob[:, :], in1=b2t[:, NH:OUTF],
                    op=Alu.add)
                eng_dma.dma_start(out=out[tok0 : tok0 + 128, NH:OUTF],
                                  in_=ob[:, :])

            # tile list: 4-row tiles, with a 3+1 split at the end so the
            # serial fc2 tail after the final fc1 is a single 128-token
            # chunk. fc2 chunks of tile t-1 are interleaved between tile
            # t's fc1 m-pairs so the PSUM-recycle / bias-add latency
            # hides behind ~5us of fc1 instead of stalling the PE at
            # every other 2us fc2 chunk boundary.
            tiles = [(i * TILE_ROWS, TILE_ROWS) for i in range(N_TILES - 1)]
            tiles += [((N_TILES - 1) * TILE_ROWS, TILE_ROWS - 1),
                      (ROWS_PER_CORE - 1, 1)]

            prev = None  # (hts, r0, tt) of the previous tile: fc2 runs one
            # tile behind fc1 so the PE never waits on the GELU latency
            for t_i, (r0, nrows) in enumerate(tiles):
                tt = nrows * WIMG

                # tile 0's chunk DMAs were emitted up top (chunk-paced
                # fc1 start); later tiles use one merged gather DMA on
                # the sync queue (13KB contiguous runs per partition)
                if t_i == 0:
                    fts = f0

                    def ftile(j, p0=0, p1=128):
                        return fts[j][p0:p1, :]
                else:
                    fsup = fpool.tile([128, TT * KC], mm_dt, tag="fs")
                    used = fsup[:, 0 : nrows * KC * WIMG]
                    dst = used.rearrange("p (r j w) -> p r j w",
                                         r=nrows, j=KC)
                    nc.sync.dma_start(out=dst, in_=xr[:, r0 : r0 + nrows, :, :])
                    fview = used.rearrange("p (r j w) -> p j r w",
                                           r=nrows, j=KC)

                    def ftile(j, p0=0, p1=128):
                        return fview[p0:p1, j]

                if t_i == 1:
                    # w2/b2 ride the sync queue behind tile-1's gathers:
                    # the sync engine runs ~3 tiles ahead of the PE, so
                    # they land during tile 0's compute — well before the
                    # first interleaved fc2 chunk (tile-1 pair-0 end).
                    # The scalar queue would serialize them behind tile
                    # 0's eight GELU ACTIVATEs instead.
                    for j2 in range(HID // 128):
                        t = wpool.tile([128, OUTF], mm_dt, tag=f"w2_{j2}")
                        nc.sync.dma_start(out=t[:, :], in_=w2t[j2, :, :])
                        w2sb.append(t)
                    b2t = wpool.tile([128, OUTF], f32, tag="b2")
                    nc.sync.dma_start(out=b2t[:, :], in_=b2bc[:, :])
                # --- fc1 + GELU: h[m] = gelu(w1p[j].T @ featT + b1) ---
                hts = []
                if t_i == 0:
                    # tile 0 is DMA-paced: loop j OUTER with all 8 PSUM
                    # banks as accumulators, so each arriving chunk feeds
                    # 8 matmuls (~1.7us) — matching the ~1.8us/chunk HBM
                    # delivery rate instead of stalling on a chunk-starved
                    # m-pair pass
                    accs = []
                    for m in range(HID // 128):
                        pool_m = (ps1, ps2)[m // 4]
                        tag_m = ("psa", "psb")[m % 2] if m < 4 else \
                            ("poa", "pob")[m % 2]
                        acc = pool_m.tile([128, tt], f32, tag=tag_m,
                                          name=f"acc{m}")
                        accs.append(acc)
                    fine_j0 = os.environ.get("KERNEL_FINE_J0", "1") == "1"
                    if fine_j0:
                        # j=0 fine-grained: token-slice outer, m inner, so
                        # MM (s,m) only needs f-slice s + w1 quarter m//2.
                        # start=True only on s=0: a start resets the whole
                        # bank's has_written bits, so later slices must
                        # write start=False (they land on has_written=0
                        # regions and store rather than accumulate).
                        for s4 in range(4):
                            for m in range(HID // 128):
                                nc.tensor.matmul(
                                    accs[m][:, s4 * 128 : (s4 + 1) * 128],
                                    w1sb[0][:, m * 128 : (m + 1) * 128],
                                    f0[0][:, s4 * 128 : (s4 + 1) * 128],
                                    start=(s4 == 0), stop=False,
                                )
                    else:
                        for m in range(HID // 128):
                            nc.tensor.matmul(
                                accs[m][:, :],
                                w1sb[0][:, m * 128 : (m + 1) * 128],
                                ftile(0),
                                start=True, stop=False,
                            )
                    for j in range(1, KC - 1):
                        for m in range(HID // 128):
                            nc.tensor.matmul(
                                accs[m][:, :],
                                w1sb[j][:, m * 128 : (m + 1) * 128],
                                ftile(j),
                                start=False, stop=False,
                            )
                    # K=64 tail chunk: m-pairs ride PE row groups
                    # (0,0)/(64,0) concurrently, like the packed path
                    for mp0 in range(HID // 256):
                        m0, m1 = 2 * mp0, 2 * mp0 + 1
                        nc.tensor.matmul(
                            accs[m0][:, :],
                            w1sb[KC - 1][0:64, m0 * 128 : (m0 + 1) * 128],
                            ftile(KC - 1, 0, 64),
                            start=False, stop=True,
                        )
                        nc.tensor.matmul(
                            accs[m1][:, :],
                            w1sb[KC - 1][64:128, m1 * 128 : (m1 + 1) * 128],
                            ftile(KC - 1, 64, 128),
                            start=False, stop=True,
                        )
                    for m in range(HID // 128):
                        ht = hpool.tile([128, tt], mm_dt, tag=f"h{m}")
                        nc.scalar.activation(ht[:, :], accs[m][:, :], ACT,
                                             bias=b1t[:, m : m + 1], scale=1.0)
                        hts.append(ht)
                elif not pack64:
                    for m in range(HID // 128):
                        ps = ps1.tile([128, tt], f32)
                        for j in range(KC):
                            kr = 128 if j < KC - 1 else 64
                            nc.tensor.matmul(
                                ps[:, :],
                                w1sb[j][0:kr, m * 128 : (m + 1) * 128],
                                ftile(j, 0, kr),
                                start=(j == 0), stop=(j == KC - 1),
                            )
                        ht = hpool.tile([128, tt], mm_dt, tag=f"h{m}")
                        nc.scalar.activation(ht[:, :], ps[:, :], ACT,
                                             bias=b1t[:, m : m + 1], scale=1.0)
                        hts.append(ht)
                    if prev is not None:
                        ph, pr0, ptt = prev
                        for s in range(ptt // 128):
                            fc2_chunk(ph, pr0, s)
                else:
                    # chunk 12 (K=64) packed: m-pairs run their K=64 matmuls
                    # concurrently on PE row groups (0,0)/(64,0).
                    # prev tile's fc2 chunks are spread over the m-pairs
                    # (floor distribution, empty-first so chunk 0 waits
                    # for at least one ~5us pair of vector/DMA slack).
                    # The K=64 tail chunk runs as row-group (h0/h64) MM
                    # pairs, but a row-group LDWEIGHTS cannot overlap an
                    # in-flight full-array matmul (and vice versa), so
                    # every full<->row-group seam costs ~100ns of
                    # unhidden LDWEIGHTS. Group the K=64 finishes of two
                    # pairs (and the fc2 block) per seam: 2 seams/tile
                    # instead of 4.
                    npair = HID // 256
                    C = prev[2] // 128 if prev is not None else 0
                    hts = [None] * (HID // 128)
                    pend_k64 = []
                    for mp in range(npair):
                        m0, m1 = 2 * mp, 2 * mp + 1
                        psa = ps1.tile([128, tt], f32, tag="psa")
                        psb = ps1.tile([128, tt], f32, tag="psb")
                        for j in range(KC - 1):
                            nc.tensor.matmul(
                                psa[:, :], w1sb[j][:, m0 * 128:(m0 + 1) * 128],
                                ftile(j), start=(j == 0), stop=False)
                            nc.tensor.matmul(
                                psb[:, :], w1sb[j][:, m1 * 128:(m1 + 1) * 128],
                                ftile(j), start=(j == 0), stop=False)
                        pend_k64.append((mp, psa, psb))
                        if mp not in (1, 3):
                            continue
                        for mq, qa, qb in pend_k64:
                            nc.tensor.matmul(
                                qa[:, :],
                                w1sb[KC - 1][0:64, 2 * mq * 128:(2 * mq + 1) * 128],
                                ftile(KC - 1, 0, 64),
                                start=False, stop=True)
                            nc.tensor.matmul(
                                qb[:, :],
                                w1sb[KC - 1][64:128, (2 * mq + 1) * 128:(2 * mq + 2) * 128],
                                ftile(KC - 1, 64, 128),
                                start=False, stop=True)
                        for mq, qa, qb in pend_k64:
                            for m, pst in ((2 * mq, qa), (2 * mq + 1, qb)):
                                ht = hpool.tile([128, tt], mm_dt, tag=f"h{m}")
                                nc.scalar.activation(
                                    ht[:, :], pst[:, :], ACT,
                                    bias=b1t[:, m : m + 1], scale=1.0)
                                hts[m] = ht
                        pend_k64 = []
                        # fc2 chunks of the prev tile ride in blocks of 2
                        # inside the same seam window
                        if prev is not None:
                            ph, pr0, _ = prev
                            half = (C + 1) // 2
                            lo, hi = (0, half) if mp == 1 else (half, C)
                            for s in range(lo, hi):
                                fc2_chunk(ph, pr0, s)
                prev = (hts, r0, tt)
            # tail: the final tile's fc2 (a single 128-token chunk)
            ph, pr0, ptt = prev
            for s in range(ptt // 128):
                fc2_chunk(ph, pr0, s, last=(s == ptt // 128 - 1))

    if os.environ.get("KERNEL_SPLITWAITS", "1") == "1":
        _split_waits(nc)
    _NC_CACHE["nc"] = nc
    return nc


def _host_prep(x, w1, b1, w2, b2):
    x = np.ascontiguousarray(np.asarray(x, dtype=np.float32))
    w1 = np.asarray(w1, dtype=np.float32)
    b1 = np.asarray(b1, dtype=np.float32)
    w2 = np.asarray(w2, dtype=np.float32)
    b2 = np.asarray(b2, dtype=np.float32)

    xe = _build_xe(x)
    w1t = np.ascontiguousarray(w1.T)  # (1600, 1024) rows c*25+k
    w1p = np.ascontiguousarray(
        w1t.reshape(C, 25, HID).transpose(1, 0, 2).reshape(INF, HID))
    # pad to (13*128, HID): rows 1600:1664 duplicate rows 1536:1600
    w1p = np.concatenate([w1p, w1p[INF - 64 : INF]], axis=0)
    w1p = w1p.reshape(KC, 128, HID)
    w2t = np.ascontiguousarray(w2.T).reshape(HID // 128, 128, OUTF)
    b1rs = np.ascontiguousarray(b1.reshape(HID // 128, 128).T)
    b2bc = np.ascontiguousarray(np.broadcast_to(b2, (128, OUTF)))

    if MODE == "bf16":
        import ml_dtypes
        xe = xe.astype(ml_dtypes.bfloat16)
        w1p = w1p.astype(ml_dtypes.bfloat16)
        w2t = w2t.astype(ml_dtypes.bfloat16)

    in_maps = []
    for cid in range(N_CORES):
        b, half = cid // 2, cid % 2
        h0 = half * ROWS_PER_CORE
        xpair = np.empty((KC, 128, ROWS_PER_CORE, WIMG), dtype=xe.dtype)
        for j in range(KC):
            for p in range(2):
                k = min(2 * j + p, 24)
                dh, dw, q = KPOS[k]
                xpair[j, p * 64 : (p + 1) * 64] = xe[
                    b, q::9,
                    1 + h0 + dh : 1 + h0 + dh + ROWS_PER_CORE,
                    1 + dw : 1 + dw + WIMG]
        xp0 = np.ascontiguousarray(xpair[:, :, 0:TILE_ROWS, :])
        xr = np.ascontiguousarray(xpair.transpose(1, 2, 0, 3))
        in_maps.append({
            "xp0": xp0, "xr": xr, "w1p": w1p, "w2t": w2t, "b1rs": b1rs,
            "b2bc": b2bc,
        })
    return in_maps


def _assemble(results):
    out = np.empty((B, NTOK, OUTF), dtype=np.float32)
    for cid in range(N_CORES):
        b, half = cid // 2, cid % 2
        t0 = half * TOK_PER_CORE
        out[b, t0 : t0 + TOK_PER_CORE, :] = results[cid]["out"]
    return out


def kernel(x, w1, b1, w2, b2, image_h, image_w):
    in_maps = _host_prep(x, w1, b1, w2, b2)
    nc = _build_bass()
    from concourse.bass_utils import run_bass_kernel_spmd
    res = run_bass_kernel_spmd(nc, in_maps, list(range(N_CORES)))
    return _assemble(res.results)

